# revision 2
# baseline (speedup 1.0000x reference)
"""Trainium2 Bass kernel for nn_Block_56968446214461 (GNN message passing block).

Data parallel over batch: B=4096 split across 8 NeuronCores (512 each).
Per-core tiling: 74 tiles of 7 batch elements (119 tokens; last tile
overlap-reads, writes only the remainder).

v2 design:
  - bf16 matmul operands everywhere (PSUM accumulation f32); residual bf16.
  - All transposes via dma_start_transpose (xbar), none on PE.
  - qkv: q,k computed F->F (no transposes), v F->T.
  - Softmax: exp (Act) -> mask mul -> ones-matmul denominator -> DVE
    reciprocal -> gpsimd partition_broadcast -> normalize U -> AV matmul.
    Attention halves = even heads (partitions 0-63) / odd heads (64-127).
  - LN rsqrt via Act Sqrt(var+eps) + DVE reciprocal (act tables visit only
    sqrt/exp/gelu sets: 4 loads/tile instead of 8).
  - lnA stats via PE ones-column matmuls in F layout (stat rows are
    partition-broadcast by gpsimd).
  - proj and gcn2-adjacency accumulate into a single PSUM tile.
  - GCN channel matmuls k-looped: y1k psum -> engine copy -> f32r adjacency.
  - No f32r DMAs (they poison xbar dma transposes); ablk converted on-chip.
"""

import os
import numpy as np
import ml_dtypes

import concourse.bacc as bacc
import concourse.bass as bass
import concourse.tile as tile
from concourse import mybir
from concourse import bass_utils

f32 = mybir.dt.float32
f32r = mybir.dt.float32r
bf16 = mybir.dt.bfloat16
AF = mybir.ActivationFunctionType
ALU = mybir.AluOpType

B, J, C = 4096, 17, 512
H, D, K = 8, 64, 3
N_CORES = 8
B_CORE = B // N_CORES
NB = 7
TOK = NB * J  # 119
CK = C // 128
EPS = 1e-5

_CACHE = {}
A2B = int(os.environ.get("K2_A2B", "3"))
SPB = int(os.environ.get("K2_SPB", "3"))
A3B = int(os.environ.get("K2_A3B", "3"))
XSQ_ENG = os.environ.get("K2_XSQ", "vector")  # pool|vector
MASK_ENG = os.environ.get("K2_MASK", "vector")  # pool|vector
SPLIT_XAP = os.environ.get("K2_SPLIT_XAP", "0") == "1"
QK_FT = os.environ.get("K2_QK_FT", "0") == "1"
PREF_STATS = int(os.environ.get("K2_PREF_STATS", "0"))  # 0 none, 1 xsq, 2 full
PE_TR = os.environ.get("K2_PE_TR", "")        # "" | "xz" (xa+z via PE)
PIN_FOLD = os.environ.get("K2_PIN_FOLD", "0") == "1"
CMIX = int(os.environ.get("K2_CMIX", "0"))


def _copy_on_act(mix, which, k):
    if mix == 0:
        return k != 1 if which in ("y1", "y2") else which == "vsb"
    if mix == 1:
        return k == 1 if which in ("y1", "y2") else which == "vsb"
    if mix == 2:
        return False
    if mix == 3:
        return True
    if mix == 4:  # y1 on act, y2 on dve, vsb act
        return which in ("y1", "vsb")
    return False


def _tiles(b_core):
    out = []
    i = 0
    while (i + 1) * NB <= b_core:
        out.append((i * NB, i * NB, NB))
        i += 1
    rem = b_core - i * NB
    if rem:
        out.append((b_core - NB, b_core - rem, rem))
    return out


def _bcast_row_ap(t_ap, offset_elems, nparts, n):
    return bass.AP(
        tensor=t_ap.tensor,
        offset=t_ap.offset + offset_elems,
        ap=[[0, nparts], [1, n]],
    )


def _build(b_core, flags):
    ln1aff = "ln1aff" in flags
    bias_on = {k for k in flags if k.startswith("b_")}

    nc = bacc.Bacc("TRN2", target_bir_lowering=False, debug=False)
    ntok = b_core * J
    assert ntok >= 128, "xbar transpose load path needs at least 128 tokens"

    # ---- DRAM I/O (no f32r DMAs!) ----
    xbd = nc.dram_tensor("xb", [ntok, C], bf16, kind="ExternalInput")
    w1d = nc.dram_tensor("w1b", [C, K * C], bf16, kind="ExternalInput")
    wqkd = nc.dram_tensor("wqkb", [C, 2 * C], bf16, kind="ExternalInput")
    wvd = nc.dram_tensor("wvb", [C, C], bf16, kind="ExternalInput")
    wpd = nc.dram_tensor("wpb", [C, C], bf16, kind="ExternalInput")
    w2d = nc.dram_tensor("w2b", [C, K * C], bf16, kind="ExternalInput")
    m1d = nc.dram_tensor("m1b", [C, 256], bf16, kind="ExternalInput")
    m2d = nc.dram_tensor("m2b", [256, 256], bf16, kind="ExternalInput")
    m3d = nc.dram_tensor("m3b", [256, C], bf16, kind="ExternalInput")
    ablkd = nc.dram_tensor("ablk", [TOK, K * TOK], f32, kind="ExternalInput")
    maskd = nc.dram_tensor("maskd", [TOK, TOK], bf16, kind="ExternalInput")
    identd = nc.dram_tensor("identd", [TOK, TOK], bf16, kind="ExternalInput")
    g1d = nc.dram_tensor("g1d", [J], f32, kind="ExternalInput")
    b1d = nc.dram_tensor("b1d", [J], f32, kind="ExternalInput")
    biasd = {}
    for nm, ln in [("b_qkv", 3 * C), ("b_p", C), ("b_1g", K * C), ("b_2g", K * C),
                   ("b_m1", 256), ("b_m2", 256), ("b_m3", C)]:
        if nm in bias_on:
            biasd[nm] = nc.dram_tensor(nm, [ln], f32, kind="ExternalInput")
    # F-layout (per-partition) bias views, host-prepped as [128, n]
    biasf = {}
    for nm, n2 in [("bf_qk", 8), ("bf_m1", 2), ("bf_m2", 2)]:
        src = {"bf_qk": "b_qkv", "bf_m1": "b_m1", "bf_m2": "b_m2"}[nm]
        if src in bias_on:
            biasf[nm] = nc.dram_tensor(nm, [128, n2], f32, kind="ExternalInput")
    outd = nc.dram_tensor("out", [ntok, C], f32, kind="ExternalOutput")

    with tile.TileContext(nc) as tc:
        with tc.tile_pool(name="const", bufs=1) as cpool, \
             tc.tile_pool(name="act2", bufs=A2B) as a2, \
             tc.tile_pool(name="act3", bufs=A3B) as a3, \
             tc.tile_pool(name="scr", bufs=SPB) as sp, \
             tc.tile_pool(name="ps_a", bufs=2, space="PSUM") as ps_a, \
             tc.tile_pool(name="ps_b", bufs=1, space="PSUM") as ps_b, \
             tc.tile_pool(name="ps_c", bufs=2, space="PSUM") as ps_c, \
             tc.tile_pool(name="ps_d", bufs=1, space="PSUM") as ps_d, \
             tc.tile_pool(name="ps_e", bufs=2, space="PSUM") as ps_e:

            # ---- one-time constants ----
            w1s = cpool.tile([128, CK, K * C], bf16)
            nc.sync.dma_start(out=w1s, in_=w1d.ap().rearrange("(c p) n -> p c n", c=CK))
            wqks = cpool.tile([128, CK, 2 * C], bf16)
            nc.sync.dma_start(out=wqks, in_=wqkd.ap().rearrange("(c p) n -> p c n", c=CK))
            wvs = cpool.tile([128, CK, C], bf16)
            nc.sync.dma_start(out=wvs, in_=wvd.ap().rearrange("(c p) n -> p c n", c=CK))
            wps = cpool.tile([128, CK, C], bf16)
            nc.sync.dma_start(out=wps, in_=wpd.ap().rearrange("(c p) n -> p c n", c=CK))
            w2s = cpool.tile([128, CK, K * C], bf16)
            nc.sync.dma_start(out=w2s, in_=w2d.ap().rearrange("(c p) n -> p c n", c=CK))
            m1s = cpool.tile([128, CK, 256], bf16)
            nc.sync.dma_start(out=m1s, in_=m1d.ap().rearrange("(c p) n -> p c n", c=CK))
            m2s = cpool.tile([128, 2, 256], bf16)
            nc.sync.dma_start(out=m2s, in_=m2d.ap().rearrange("(c p) n -> p c n", c=2))
            m3s = cpool.tile([128, 2, C], bf16)
            nc.sync.dma_start(out=m3s, in_=m3d.ap().rearrange("(c p) n -> p c n", c=2))
            ablkf = cpool.tile([TOK, K, TOK], f32, tag="ablkf")
            nc.sync.dma_start(out=ablkf, in_=ablkd.ap().rearrange("p (k w) -> p k w", k=K))
            ablk = cpool.tile([TOK, K, TOK], f32r, tag="ablk32")
            nc.scalar.copy(out=ablk, in_=ablkf)
            mask = cpool.tile([TOK, TOK], bf16, tag="maskb")
            nc.sync.dma_start(out=mask, in_=maskd.ap())
            ident = cpool.tile([TOK, TOK], bf16, tag="identb")
            nc.sync.dma_start(out=ident, in_=identd.ap())
            epst = cpool.tile([128, 1], f32, tag="epst")
            nc.vector.memset(epst, EPS)
            if ln1aff:
                g1t = cpool.tile([128, J], f32, tag="g1t")
                nc.sync.dma_start(out=g1t, in_=_bcast_row_ap(g1d.ap(), 0, 128, J))
                b1t = cpool.tile([128, J], f32, tag="b1t")
                nc.sync.dma_start(out=b1t, in_=_bcast_row_ap(b1d.ap(), 0, 128, J))
            btiles = {}
            for nm, t in biasd.items():
                ln = t.shape[0]
                bt = cpool.tile([128, ln], f32, tag=f"bt_{nm}")
                nc.sync.dma_start(out=bt, in_=_bcast_row_ap(t.ap(), 0, 128, ln))
                btiles[nm] = bt
            bftiles = {}
            for nm, t in biasf.items():
                n2 = t.shape[1]
                bt = cpool.tile([128, n2], f32, tag=f"bt_{nm}")
                nc.sync.dma_start(out=bt, in_=t.ap())
                bftiles[nm] = bt

            xba = xbd.ap()
            outa = outd.ap()
            scl = float(D) ** -0.5

            def emit_loads(t0):
                """x (T layout, bf16) + xbar transpose (F layout)."""
                t0x = min(t0, ntok - 128)
                xTb = a3.tile([TOK, C], bf16, tag="xTb")
                nc.sync.dma_start(out=xTb, in_=xba[t0:t0 + TOK, :])
                xF = a3.tile([128, CK, 128], bf16, tag="xF")
                nc.sync.dma_start_transpose(out=xF, in_=xba[t0x:t0x + 128, :])
                return xTb, xF, t0 - t0x

            def emit_xsq(ld):
                xTb, xF, co = ld
                xFv = xF[:, :, co:co + TOK]
                xsq = a2.tile([128, CK, TOK], bf16, tag="xsq")
                (nc.gpsimd if XSQ_ENG == "pool" else nc.vector).tensor_mul(
                    out=xsq, in0=xFv, in1=xFv)
                return xsq

            def emit_stats(ld, xsq):
                """LN1 + lnA statistics for a tile (pipelined one tile ahead)."""
                xTb, xF, co = ld
                xFv = xF[:, :, co:co + TOK]
                xFg = xFv.rearrange("p c (b j) -> p c b j", j=J)
                s1 = sp.tile([128, CK, NB], f32, tag="s1")
                nc.vector.tensor_reduce(out=s1, in_=xFg, axis=mybir.AxisListType.X,
                                        op=ALU.add)
                s2 = sp.tile([128, CK, NB], f32, tag="s2")
                nc.vector.tensor_reduce(out=s2,
                                        in_=xsq.rearrange("p c (b j) -> p c b j", j=J),
                                        axis=mybir.AxisListType.X, op=ALU.add)
                aa = sp.tile([128, CK, NB], f32, tag="aa")
                nc.vector.tensor_mul(out=aa, in0=s1, in1=s1)
                bb_ = sp.tile([128, CK, NB], f32, tag="bb_")
                nc.vector.scalar_tensor_tensor(out=bb_, in0=aa, scalar=1.0 / J,
                                               in1=s2, op0=ALU.mult, op1=ALU.subtract)
                sdj = sp.tile([128, CK, NB], f32, tag="sdj")
                nc.scalar.activation(out=sdj, in_=bb_, func=AF.Sqrt,
                                     bias=epst, scale=-1.0 / J)
                rj = sp.tile([128, CK, NB], f32, tag="rj")
                nc.vector.reciprocal(out=rj, in_=sdj)
                mrj = sp.tile([128, CK, NB], f32, tag="mrj")
                nc.vector.scalar_tensor_tensor(out=mrj, in0=s1, scalar=1.0 / J,
                                               in1=rj, op0=ALU.mult, op1=ALU.mult)
                stA = sp.tile([TOK, 6], f32, tag="stA")
                nc.vector.bn_stats(out=stA, in_=xTb)
                mvA = sp.tile([TOK, 2], f32, tag="mvA")
                nc.vector.bn_aggr(out=mvA, in_=stA)
                sdA = sp.tile([TOK, 1], f32, tag="sdA")
                nc.scalar.activation(out=sdA, in_=mvA[:, 1:2], func=AF.Sqrt,
                                     bias=epst[:TOK], scale=1.0)
                rA = sp.tile([TOK, 1], f32, tag="rA")
                nc.vector.reciprocal(out=rA, in_=sdA)
                return rj, mrj, mvA, rA

            tls = _tiles(b_core)
            loads = {0: emit_loads(tls[0][0] * J)}
            xsqs = {0: emit_xsq(loads[0])}
            stats = {0: emit_stats(loads[0], xsqs[0])}

            for ti, (b0, wb0, wnb) in enumerate(tls):
                t0 = b0 * J
                woff = (wb0 - b0) * J
                wntok = wnb * J
                # prefetch next tile's loads + stats ahead of this tile's
                # compute so the in-order queues never park them behind
                # late-phase work of the current tile
                if ti + 1 < len(tls):
                    loads[ti + 1] = emit_loads(tls[ti + 1][0] * J)
                    if PREF_STATS >= 1:
                        xsqs[ti + 1] = emit_xsq(loads[ti + 1])
                    if PREF_STATS >= 2:
                        stats[ti + 1] = emit_stats(loads[ti + 1], xsqs[ti + 1])
                xTb, xF, co = loads.pop(ti)
                if ti not in xsqs:
                    xsqs[ti] = emit_xsq(loads[ti] if ti in loads else (xTb, xF, co))
                if ti not in stats:
                    stats[ti] = emit_stats((xTb, xF, co), xsqs[ti])
                rj, mrj, mvA, rA = stats.pop(ti)
                xsqs.pop(ti, None)
                xFv = xF[:, :, co:co + TOK]
                xFg = xFv.rearrange("p c (b j) -> p c b j", j=J)

                xg = a2.tile([128, CK, TOK], bf16, tag="xg")
                xgg = xg.rearrange("p c (b j) -> p c b j", j=J)
                tmp1 = sp.tile([128, CK, TOK], bf16, tag="tmp1")
                t1g = tmp1.rearrange("p c (b j) -> p c b j", j=J)
                nc.vector.tensor_mul(out=t1g, in0=xFg,
                                     in1=rj.to_broadcast([128, CK, NB, J]))
                nc.vector.tensor_sub(out=xgg, in0=t1g,
                                     in1=mrj.to_broadcast([128, CK, NB, J]))
                if ln1aff:
                    ga = g1t
                    gb = bass.AP(tensor=ga.tensor, offset=ga.offset,
                                 ap=[ga.ap[0], [0, CK], [0, NB], ga.ap[1]])
                    ba = b1t
                    bb = bass.AP(tensor=ba.tensor, offset=ba.offset,
                                 ap=[ba.ap[0], [0, CK], [0, NB], ba.ap[1]])
                    nc.vector.tensor_mul(out=xgg, in0=xgg, in1=gb)
                    nc.vector.tensor_add(out=xgg, in0=xgg, in1=bb)

                # ---- lnA apply + xbar transpose -> xaF ----
                xa = a2.tile([128, C], bf16, tag="xa")
                nc.vector.tensor_scalar(out=xa[:TOK, :], in0=xTb, scalar1=mvA[:, 0:1],
                                        scalar2=rA, op0=ALU.subtract, op1=ALU.mult)
                if "x" in PE_TR:
                    xatp = ps_e.tile([128, CK, 120], bf16, tag="pse")
                    for c in range(CK):
                        nc.tensor.transpose(xatp[:, c, :TOK],
                                            xa[:TOK, c * 128:(c + 1) * 128], ident)
                    xaFt = a2.tile([128, CK, TOK], bf16, tag="xaFt")
                    nc.vector.tensor_copy(out=xaFt, in_=xatp[:, :, :TOK])
                    xaF = xaFt
                else:
                    xaFt = a2.tile([128, CK, 128], bf16, tag="xaFt")
                    nc.sync.dma_start_transpose(out=xaFt, in_=xa)
                    xaF = xaFt[:, :, :TOK]

                # ---- GCN1: k-loop channel matmul + adjacency ----
                xg1p = ps_b.tile([TOK, C], f32, tag="psb")
                for k in range(K):
                    y1p = ps_a.tile([TOK, C], f32, tag="psa")
                    for c in range(CK):
                        nc.tensor.matmul(y1p, xg[:, c, :],
                                         w1s[:, c, k * C:(k + 1) * C],
                                         start=(c == 0), stop=(c == CK - 1))
                    y1s = sp.tile([TOK, C], f32r, tag=f"y1s{k}")
                    if "b_1g" in bias_on:
                        nc.vector.tensor_add(out=y1s, in0=y1p,
                                             in1=btiles["b_1g"][:TOK, k * C:(k + 1) * C])
                    elif _copy_on_act(CMIX, "y1", k):
                        nc.scalar.copy(out=y1s, in_=y1p)
                    else:
                        nc.vector.tensor_copy(out=y1s, in_=y1p)
                    nc.tensor.matmul(xg1p, ablk[:, k, :], y1s,
                                     start=(k == 0), stop=(k == K - 1))

                # xg1 -> bf16 -> xbar transpose -> xg1F
                xg1c = a2.tile([128, C], bf16, tag="xg1c")
                if PIN_FOLD:
                    nc.vector.tensor_scalar_mul(out=xg1c[:TOK, :], in0=xg1p,
                                                scalar1=0.5)
                else:
                    nc.vector.tensor_copy(out=xg1c[:TOK, :], in_=xg1p)
                xg1Ft = a2.tile([128, CK, 128], bf16, tag="xg1Ft")
                nc.sync.dma_start_transpose(out=xg1Ft, in_=xg1c)
                xg1F = xg1Ft[:, :, :TOK]

                # ---- qkv: q,k ----
                if QK_FT:
                    qkTc = a2.tile([128, 2 * C], bf16, tag="qkTc")
                    for g in range(2):
                        qkp = ps_c.tile([TOK, C], f32, tag="psc")
                        for c in range(CK):
                            nc.tensor.matmul(qkp, xaF[:, c, :],
                                             wqks[:, c, g * C:(g + 1) * C],
                                             start=(c == 0), stop=(c == CK - 1))
                        dst = qkTc[:TOK, g * C:(g + 1) * C]
                        if "b_qkv" in bias_on:
                            nc.vector.tensor_add(out=dst, in0=qkp,
                                                 in1=btiles["b_qkv"][:TOK, g * C:(g + 1) * C])
                        elif g == 0:
                            nc.vector.tensor_copy(out=dst, in_=qkp)
                        else:
                            nc.scalar.copy(out=dst, in_=qkp)
                    qkFt = a2.tile([128, 8, 128], bf16, tag="qkFt")
                    nc.sync.dma_start_transpose(out=qkFt, in_=qkTc)
                    qkF = qkFt[:, :, :TOK]
                else:
                    qkFf = a2.tile([128, 8, TOK], bf16, tag="qkF")
                    for g in range(2):
                        qkp = ps_c.tile([128, 4, TOK], f32, tag="psc")
                        for d in range(4):
                            dd = g * 4 + d
                            for c in range(CK):
                                nc.tensor.matmul(qkp[:, d, :], wqks[:, c, dd * 128:(dd + 1) * 128],
                                                 xaF[:, c, :], start=(c == 0), stop=(c == CK - 1))
                        dst = qkFf[:, g * 4:(g + 1) * 4, :]
                        if "b_qkv" in bias_on:
                            bq = bftiles["bf_qk"]
                            bqa = bass.AP(tensor=bq.tensor,
                                          offset=bq.offset + g * 4,
                                          ap=[bq.ap[0], [1, 4], [0, TOK]])
                            nc.vector.tensor_add(out=dst, in0=qkp, in1=bqa)
                        elif g == 0:
                            nc.vector.tensor_copy(out=dst, in_=qkp)
                        else:
                            nc.scalar.copy(out=dst, in_=qkp)
                    qkF = qkFf

                # ---- v: F->T ----
                vp = ps_c.tile([TOK, C], f32, tag="psc")
                for c in range(CK):
                    nc.tensor.matmul(vp, xaF[:, c, :], wvs[:, c, :],
                                     start=(c == 0), stop=(c == CK - 1))
                vsb = a2.tile([TOK, H, 65], bf16, tag="vsb")
                nc.vector.memset(vsb[:, :, 64:65], 1.0)
                vdst = vsb[:, :, 0:64]
                vsrc = vp.rearrange("p (h d) -> p h d", h=H)
                if "b_qkv" in bias_on:
                    bv = btiles["b_qkv"][:TOK, 2 * C:3 * C].rearrange(
                        "p (h d) -> p h d", h=H)
                    nc.vector.tensor_add(out=vdst, in0=vsrc, in1=bv)
                elif _copy_on_act(CMIX, "vsb", 0):
                    nc.scalar.copy(out=vdst, in_=vsrc)
                else:
                    nc.vector.tensor_copy(out=vdst, in_=vsrc)

                # ---- attention: halves = even heads (part 0-63) / odd ----
                oF = a2.tile([128, CK, TOK], bf16, tag="oF")
                for half in range(2):
                    p0 = half * 64
                    scp = ps_c.tile([TOK, 4, TOK], f32, tag="psc")
                    for j in range(4):
                        nc.tensor.matmul(scp[:, j, :], qkF[p0:p0 + 64, 4 + j, :],
                                         qkF[p0:p0 + 64, j, :], start=True, stop=True)
                    U = a2.tile([TOK, 4, TOK], bf16, tag=f"U{half}")
                    nc.scalar.activation(out=U, in_=scp, func=AF.Exp, scale=scl)
                    mb = bass.AP(tensor=mask.tensor, offset=mask.offset,
                                 ap=[mask.ap[0], [0, 4], mask.ap[1]])
                    (nc.gpsimd if MASK_ENG == "pool" else nc.vector).tensor_mul(
                        out=U, in0=U, in1=mb)
                    ozp = ps_c.tile([65, 4, TOK], f32, tag="psc")
                    for j in range(4):
                        h = 2 * j + half
                        nc.tensor.matmul(ozp[:, j, :], vsb[:, h, :], U[:, j, :],
                                         start=True, stop=True)
                    rz = sp.tile([1, 4, TOK], bf16, tag=f"rz{half}")
                    with nc.allow_low_precision(reason="bf16 softmax recip ok"):
                        nc.vector.reciprocal(out=rz, in_=ozp[64:65, :, :])
                    rzb = sp.tile([64, 4, TOK], bf16, tag=f"rzb{half}")
                    nc.gpsimd.partition_broadcast(rzb, rz)
                    if half == 0:
                        nc.vector.tensor_mul(out=oF[p0:p0 + 64, :, :],
                                             in0=ozp[0:64, :, :], in1=rzb)
                    else:
                        nc.vector.tensor_mul(out=oF[p0:p0 + 64, :, :],
                                             in0=ozp[0:64, :, :], in1=rzb)

                # ---- pin/gin (F layout) ----
                xg1Fv = xg1F
                if PIN_FOLD:
                    # xg1F already holds 0.5*xg1; gin = (2.5*xg1F_half + oF),
                    # with the 0.8 folded into w2 on the host.
                    gin = a2.tile([128, CK, TOK], bf16, tag="gin")
                    nc.vector.scalar_tensor_tensor(out=gin, in0=xg1Fv, scalar=2.5,
                                                   in1=oF, op0=ALU.mult, op1=ALU.add)
                else:
                    pin = a2.tile([128, CK, TOK], bf16, tag="pin")
                    nc.vector.scalar_tensor_tensor(out=pin, in0=xg1Fv, scalar=0.5,
                                                   in1=oF, op0=ALU.mult, op1=ALU.add)
                    gin = a2.tile([128, CK, TOK], bf16, tag="gin")
                    nc.vector.scalar_tensor_tensor(out=gin, in0=oF, scalar=0.8,
                                                   in1=xg1Fv, op0=ALU.mult, op1=ALU.add)

                # ---- proj (+ gcn2 adjacency accumulated or split) ----
                xap = ps_b.tile([TOK, C], f32, tag="psb")
                if PIN_FOLD:
                    for c in range(CK):
                        nc.tensor.matmul(xap, oF[:, c, :], wps[:, c, :],
                                         start=(c == 0), stop=False)
                    for c in range(CK):
                        nc.tensor.matmul(xap, xg1Fv[:, c, :], wps[:, c, :],
                                         start=False, stop=SPLIT_XAP and c == CK - 1)
                else:
                    for c in range(CK):
                        nc.tensor.matmul(xap, pin[:, c, :], wps[:, c, :],
                                         start=(c == 0), stop=SPLIT_XAP and c == CK - 1)

                yT = a3.tile([TOK, C], f32, tag="yT")
                if SPLIT_XAP:
                    nc.vector.tensor_add(out=yT, in0=xap, in1=xTb)
                if SPLIT_XAP:
                    xg2p = ps_b.tile([TOK, C], f32, tag="psb")
                else:
                    xg2p = xap
                for k in range(K):
                    y2p = ps_e.tile([TOK, C], f32, tag="pse")
                    for c in range(CK):
                        nc.tensor.matmul(y2p, gin[:, c, :],
                                         w2s[:, c, k * C:(k + 1) * C],
                                         start=(c == 0), stop=(c == CK - 1))
                    y2s = sp.tile([TOK, C], f32r, tag=f"y2s{k}")
                    if "b_2g" in bias_on:
                        nc.vector.tensor_add(out=y2s, in0=y2p,
                                             in1=btiles["b_2g"][:TOK, k * C:(k + 1) * C])
                    elif _copy_on_act(CMIX, "y2", k):
                        nc.scalar.copy(out=y2s, in_=y2p)
                    else:
                        nc.vector.tensor_copy(out=y2s, in_=y2p)
                    nc.tensor.matmul(xg2p, ablk[:, k, :], y2s,
                                     start=SPLIT_XAP and k == 0, stop=(k == K - 1))

                # ---- y = x + proj + gcn2 [+ proj bias] ----
                if SPLIT_XAP:
                    nc.vector.tensor_add(out=yT, in0=xg2p, in1=yT)
                else:
                    nc.vector.tensor_add(out=yT, in0=xap, in1=xTb)
                if "b_p" in bias_on:
                    nc.vector.tensor_add(out=yT, in0=yT, in1=btiles["b_p"][:TOK, :])

                # ---- LN2 + transpose ----
                st2 = sp.tile([TOK, 6], f32, tag="st2")
                nc.vector.bn_stats(out=st2, in_=yT)
                mv2 = sp.tile([TOK, 2], f32, tag="mv2")
                nc.vector.bn_aggr(out=mv2, in_=st2)
                sd2 = sp.tile([TOK, 1], f32, tag="sd2")
                nc.scalar.activation(out=sd2, in_=mv2[:, 1:2], func=AF.Sqrt,
                                     bias=epst[:TOK], scale=1.0)
                r2 = sp.tile([TOK, 1], f32, tag="r2")
                nc.vector.reciprocal(out=r2, in_=sd2)
                z = a2.tile([128, C], bf16, tag="z")
                nc.vector.tensor_scalar(out=z[:TOK, :], in0=yT, scalar1=mv2[:, 0:1],
                                        scalar2=r2, op0=ALU.subtract, op1=ALU.mult)
                if "z" in PE_TR:
                    ztp = ps_e.tile([128, CK, 120], bf16, tag="pse")
                    for c in range(CK):
                        nc.tensor.transpose(ztp[:, c, :TOK],
                                            z[:TOK, c * 128:(c + 1) * 128], ident)
                    zFt = a2.tile([128, CK, TOK], bf16, tag="zFt")
                    nc.scalar.copy(out=zFt, in_=ztp[:, :, :TOK])
                    zFv = zFt
                else:
                    zFt = a2.tile([128, CK, 128], bf16, tag="zFt")
                    nc.sync.dma_start_transpose(out=zFt, in_=z)
                    zFv = zFt[:, :, :TOK]

                # ---- MLP: m1 F->F, m2 F->F, m3 F->T ----
                h1p = ps_d.tile([128, 2, TOK], f32, tag="psd")
                for d in range(2):
                    for c in range(CK):
                        nc.tensor.matmul(h1p[:, d, :], m1s[:, c, d * 128:(d + 1) * 128],
                                         zFv[:, c, :], start=(c == 0), stop=(c == CK - 1))
                h1F = a2.tile([128, 2, TOK], bf16, tag="h1F")
                if "b_m1" in bias_on:
                    bm1 = bftiles["bf_m1"]
                    for d in range(2):
                        nc.scalar.activation(out=h1F[:, d, :], in_=h1p[:, d, :],
                                             func=AF.Gelu,
                                             bias=bm1[:, d:d + 1], scale=1.0)
                else:
                    nc.scalar.activation(out=h1F, in_=h1p, func=AF.Gelu)

                h2p = ps_d.tile([128, 2, TOK], f32, tag="psd")
                for d in range(2):
                    for c in range(2):
                        nc.tensor.matmul(h2p[:, d, :], m2s[:, c, d * 128:(d + 1) * 128],
                                         h1F[:, c, :], start=(c == 0), stop=(c == 1))
                g2F = a2.tile([128, 2, TOK], bf16, tag="g2F")
                if "b_m2" in bias_on:
                    bm2 = bftiles["bf_m2"]
                    for d in range(2):
                        nc.scalar.activation(out=g2F[:, d, :], in_=h2p[:, d, :],
                                             func=AF.Gelu,
                                             bias=bm2[:, d:d + 1], scale=1.0)
                else:
                    nc.scalar.activation(out=g2F, in_=h2p, func=AF.Gelu)
                h2F = a2.tile([128, 2, TOK], bf16, tag="h2F")
                nc.vector.tensor_add(out=h2F, in0=g2F, in1=h1F)

                h3p = ps_d.tile([TOK, C], f32, tag="psd")
                for c in range(2):
                    nc.tensor.matmul(h3p, h2F[:, c, :], m3s[:, c, :],
                                     start=(c == 0), stop=(c == 1))
                g3 = sp.tile([TOK, C], f32, tag="g3")
                if "b_m3" in bias_on:
                    tb3 = sp.tile([TOK, C], f32, tag="tb3")
                    nc.vector.tensor_add(out=tb3, in0=h3p, in1=btiles["b_m3"][:TOK, :])
                    nc.scalar.activation(out=g3, in_=tb3, func=AF.Gelu)
                else:
                    nc.scalar.activation(out=g3, in_=h3p, func=AF.Gelu)
                outT = a3.tile([TOK, C], f32, tag="outT")
                nc.vector.tensor_add(out=outT, in0=g3, in1=yT)

                nc.sync.dma_start(out=outa[t0 + woff:t0 + woff + wntok, :],
                                  in_=outT[woff:woff + wntok, :])

    nc.compile()
    return nc


def _is_ones(a):
    return bool(np.all(a == 1.0))


def _is_zeros(a):
    return bool(np.all(a == 0.0))


def _prep(inputs):
    """Host-side folds and layout transforms. Returns (flags, shared arrays)."""
    adj = inputs["adj"].astype(np.float32)
    f64 = np.float64

    lnA_g, lnA_b = inputs["lnA_g"], inputs["lnA_b"]
    qkv_w = inputs["qkv_w"].astype(f64)
    wqkv = (qkv_w * lnA_g.astype(f64)[None, :])
    bqkv = inputs["qkv_b"].astype(f64) + qkv_w @ lnA_b.astype(f64)

    ln2_g, ln2_b = inputs["ln2_g"], inputs["ln2_b"]
    m1_w = inputs["m1_w"].astype(f64)
    wm1 = m1_w * ln2_g.astype(f64)[None, :]
    bm1 = inputs["m1_b"].astype(f64) + m1_w @ ln2_b.astype(f64)

    flags = set()
    if not (_is_ones(inputs["ln1_g"]) and _is_zeros(inputs["ln1_b"])):
        flags.add("ln1aff")

    bf = ml_dtypes.bfloat16
    wqkvT = wqkv.astype(np.float32).T  # [C, 3C]
    shared = {
        "w1b": np.ascontiguousarray(inputs["gcn1_w"].astype(np.float32).T).astype(bf),
        "wqkb": np.ascontiguousarray(wqkvT[:, 0:2 * C]).astype(bf),
        "wvb": np.ascontiguousarray(wqkvT[:, 2 * C:3 * C]).astype(bf),
        "wpb": np.ascontiguousarray(inputs["proj_w"].astype(np.float32).T).astype(bf),
        "w2b": np.ascontiguousarray(
            (0.8 if PIN_FOLD else 1.0) * inputs["gcn2_w"].astype(np.float32).T
        ).astype(bf),
        "m1b": np.ascontiguousarray(wm1.astype(np.float32).T).astype(bf),
        "m2b": np.ascontiguousarray(inputs["m2_w"].astype(np.float32).T).astype(bf),
        "m3b": np.ascontiguousarray(inputs["m3_w"].astype(np.float32).T).astype(bf),
        "g1d": inputs["ln1_g"].astype(np.float32),
        "b1d": inputs["ln1_b"].astype(np.float32),
    }
    ablk = np.zeros((TOK, K, TOK), np.float32)
    for k in range(K):
        for b in range(NB):
            ablk[b * J:(b + 1) * J, k, b * J:(b + 1) * J] = adj[k]
    shared["ablk"] = ablk.reshape(TOK, K * TOK)
    m = np.zeros((TOK, TOK), np.float32)
    for b in range(NB):
        m[b * J:(b + 1) * J, b * J:(b + 1) * J] = 1.0
    shared["maskd"] = m.astype(bf)
    shared["identd"] = np.eye(TOK, dtype=np.float32).astype(bf)

    for nm, arr in [("b_qkv", bqkv.astype(np.float32)),
                    ("b_p", inputs["proj_b"].astype(np.float32)),
                    ("b_1g", inputs["gcn1_b"].astype(np.float32)),
                    ("b_2g", inputs["gcn2_b"].astype(np.float32)),
                    ("b_m1", bm1.astype(np.float32)),
                    ("b_m2", inputs["m2_b"].astype(np.float32)),
                    ("b_m3", inputs["m3_b"].astype(np.float32))]:
        if not _is_zeros(arr):
            flags.add(nm)
            shared[nm] = arr
    if "b_qkv" in flags:
        shared["bf_qk"] = np.ascontiguousarray(
            bqkv[:2 * C].astype(np.float32).reshape(8, 128).T)
    if "b_m1" in flags:
        shared["bf_m1"] = np.ascontiguousarray(
            bm1.astype(np.float32).reshape(2, 128).T)
    if "b_m2" in flags:
        shared["bf_m2"] = np.ascontiguousarray(
            inputs["m2_b"].astype(np.float32).reshape(2, 128).T)
    return frozenset(flags), shared


def kernel(**inputs):
    flags, shared = _prep(inputs)
    key = (B_CORE, flags)
    if key not in _CACHE:
        _CACHE[key] = _build(B_CORE, flags)
    nc = _CACHE[key]

    xb = np.ascontiguousarray(inputs["x"], dtype=np.float32) \
        .astype(ml_dtypes.bfloat16)
    in_maps = []
    for c in range(N_CORES):
        m = dict(shared)
        m["xb"] = xb[c * B_CORE:(c + 1) * B_CORE].reshape(B_CORE * J, C)
        in_maps.append(m)

    res = bass_utils.run_bass_kernel_spmd(nc, in_maps, core_ids=list(range(N_CORES)))
    outs = [res.results[c]["out"].reshape(B_CORE, J, C) for c in range(N_CORES)]
    return np.concatenate(outs, axis=0)


# revision 6
# speedup vs baseline: 1.0605x; 1.0605x over previous
"""Trainium2 Bass kernel for nn_Block_56968446214461 (GNN message passing block).

Data parallel over batch: B=4096 split across 8 NeuronCores (512 each).
Per-core tiling: 74 tiles of 7 batch elements (119 tokens; last tile
overlap-reads, writes only the remainder).

Design notes (v2, ~4.3x over the fp32 v1):
  - bf16 matmul operands everywhere (PSUM accumulates f32); bf16 residual.
  - Layout transposes via xbar dma_start_transpose on padded [128, 512]
    bf16 tiles (out[p,c,t] = in[t, c*128+p]); no PE transposes.
    NOTE: any f32r-typed DMA poisons subsequent xbar transposes, so all
    DRAM I/O is f32/bf16 and f32r constants are converted on-chip.
  - q,k computed F->F (output lands directly in per-head F layout);
    v computed F->T for the AV stationary.
  - Softmax: scores -> SBUF copy -> gpsimd pow(e^scale, s) == exp (vpowf)
    -> mask mul -> AV matmul with a fused ones-column denominator row ->
    DVE reciprocal -> gpsimd partition_broadcast -> normalize.
  - All LN rsqrts via gpsimd pow(var, -0.5): the Activation engine runs
    ONLY Gelu + dtype-cast copies (Copy lives in every activation table
    set), eliminating the 1.28us-per-switch act-table thrash entirely.
  - GCN channel matmuls k-looped through 1-bank PSUM tiles; adjacency
    contraction on f32r copies; proj + gcn2-adjacency accumulate into a
    single PSUM bank.
  - Loads for tile i+1 (x + its xbar transpose) are emitted before tile
    i's compute so the in-order SP queue never parks them behind
    late-waiting stores.
  - PSUM bank plan (8 banks) chosen so early-phase tiles of tile i+1
    never wait on late-phase consumers of tile i.
  - Engine assignment tuned against the TimelineSim cost model: GCN
    PSUM->SBUF copies on Act, stats/elementwise on DVE, pow/broadcasts
    on gpsimd.
"""

import os
import numpy as np
import ml_dtypes

import concourse.bacc as bacc
import concourse.bass as bass
import concourse.tile as tile
from concourse import mybir
from concourse import bass_utils

f32 = mybir.dt.float32
f32r = mybir.dt.float32r
bf16 = mybir.dt.bfloat16
AF = mybir.ActivationFunctionType
ALU = mybir.AluOpType

B, J, C = 4096, 17, 512
H, D, K = 8, 64, 3
N_CORES = 8
B_CORE = B // N_CORES
NB = 7
TOK = NB * J  # 119
CK = C // 128
EPS = 1e-5

_CACHE = {}
A2B = int(os.environ.get("K2_A2B", "4"))
SPB = int(os.environ.get("K2_SPB", "3"))
A3B = int(os.environ.get("K2_A3B", "3"))
XSQ_ENG = os.environ.get("K2_XSQ", "vector")  # pool|vector
MASK_ENG = os.environ.get("K2_MASK", "vector")  # pool|vector
SPLIT_XAP = os.environ.get("K2_SPLIT_XAP", "0") == "1"
QK_FT = os.environ.get("K2_QK_FT", "0") == "1"
PREF_STATS = int(os.environ.get("K2_PREF_STATS", "0"))  # 0 none, 1 xsq, 2 full
PE_TR = os.environ.get("K2_PE_TR", "")        # "" | "xz" (xa+z via PE)
PIN_FOLD = os.environ.get("K2_PIN_FOLD", "0") == "1"
CMIX = int(os.environ.get("K2_CMIX", "3"))
VP_POOL = os.environ.get("K2_VP", "c")    # c | a
H3_POOL = os.environ.get("K2_H3", "d")    # d | b
SCS_DVE = os.environ.get("K2_SCS_DVE", "0") == "1"
XG1C_ACT = os.environ.get("K2_XG1C", "dve") == "act"


def _copy_on_act(mix, which, k):
    if mix == 0:
        return k != 1 if which in ("y1", "y2") else which == "vsb"
    if mix == 1:
        return k == 1 if which in ("y1", "y2") else which == "vsb"
    if mix == 2:
        return False
    if mix == 3:
        return True
    if mix == 4:  # y1 on act, y2 on dve, vsb act
        return which in ("y1", "vsb")
    return False


def _tiles(b_core):
    out = []
    i = 0
    while (i + 1) * NB <= b_core:
        out.append((i * NB, i * NB, NB))
        i += 1
    rem = b_core - i * NB
    if rem:
        out.append((b_core - NB, b_core - rem, rem))
    return out


def _bcast_row_ap(t_ap, offset_elems, nparts, n):
    return bass.AP(
        tensor=t_ap.tensor,
        offset=t_ap.offset + offset_elems,
        ap=[[0, nparts], [1, n]],
    )


def _build(b_core, flags):
    ln1aff = "ln1aff" in flags
    bias_on = {k for k in flags if k.startswith("b_")}

    nc = bacc.Bacc("TRN2", target_bir_lowering=False, debug=False)
    ntok = b_core * J
    assert ntok >= 128, "xbar transpose load path needs at least 128 tokens"

    # ---- DRAM I/O (no f32r DMAs!) ----
    xbd = nc.dram_tensor("xb", [ntok, C], bf16, kind="ExternalInput")
    w1d = nc.dram_tensor("w1b", [C, K * C], bf16, kind="ExternalInput")
    wqkd = nc.dram_tensor("wqkb", [C, 2 * C], bf16, kind="ExternalInput")
    wvd = nc.dram_tensor("wvb", [C, C], bf16, kind="ExternalInput")
    wpd = nc.dram_tensor("wpb", [C, C], bf16, kind="ExternalInput")
    w2d = nc.dram_tensor("w2b", [C, K * C], bf16, kind="ExternalInput")
    m1d = nc.dram_tensor("m1b", [C, 256], bf16, kind="ExternalInput")
    m2d = nc.dram_tensor("m2b", [256, 256], bf16, kind="ExternalInput")
    m3d = nc.dram_tensor("m3b", [256, C], bf16, kind="ExternalInput")
    ablkd = nc.dram_tensor("ablk", [TOK, K * TOK], f32, kind="ExternalInput")
    maskd = nc.dram_tensor("maskd", [TOK, TOK], bf16, kind="ExternalInput")
    identd = nc.dram_tensor("identd", [TOK, TOK], bf16, kind="ExternalInput")
    g1d = nc.dram_tensor("g1d", [J], f32, kind="ExternalInput")
    b1d = nc.dram_tensor("b1d", [J], f32, kind="ExternalInput")
    biasd = {}
    for nm, ln in [("b_qkv", 3 * C), ("b_p", C), ("b_1g", K * C), ("b_2g", K * C),
                   ("b_m1", 256), ("b_m2", 256), ("b_m3", C)]:
        if nm in bias_on:
            biasd[nm] = nc.dram_tensor(nm, [ln], f32, kind="ExternalInput")
    # F-layout (per-partition) bias views, host-prepped as [128, n]
    biasf = {}
    for nm, n2 in [("bf_qk", 8), ("bf_m1", 2), ("bf_m2", 2)]:
        src = {"bf_qk": "b_qkv", "bf_m1": "b_m1", "bf_m2": "b_m2"}[nm]
        if src in bias_on:
            biasf[nm] = nc.dram_tensor(nm, [128, n2], f32, kind="ExternalInput")
    outd = nc.dram_tensor("out", [ntok, C], f32, kind="ExternalOutput")

    with tile.TileContext(nc) as tc:
        with tc.tile_pool(name="const", bufs=1) as cpool, \
             tc.tile_pool(name="act2", bufs=A2B) as a2, \
             tc.tile_pool(name="act3", bufs=A3B) as a3, \
             tc.tile_pool(name="scr", bufs=SPB) as sp, \
             tc.tile_pool(name="ps_a", bufs=2, space="PSUM") as ps_a, \
             tc.tile_pool(name="ps_b", bufs=1, space="PSUM") as ps_b, \
             tc.tile_pool(name="ps_c", bufs=2, space="PSUM") as ps_c, \
             tc.tile_pool(name="ps_d", bufs=1, space="PSUM") as ps_d, \
             tc.tile_pool(name="ps_e", bufs=2, space="PSUM") as ps_e:

            # ---- one-time constants ----
            w1s = cpool.tile([128, CK, K * C], bf16)
            nc.sync.dma_start(out=w1s, in_=w1d.ap().rearrange("(c p) n -> p c n", c=CK))
            wqks = cpool.tile([128, CK, 2 * C], bf16)
            nc.sync.dma_start(out=wqks, in_=wqkd.ap().rearrange("(c p) n -> p c n", c=CK))
            wvs = cpool.tile([128, CK, C], bf16)
            nc.sync.dma_start(out=wvs, in_=wvd.ap().rearrange("(c p) n -> p c n", c=CK))
            wps = cpool.tile([128, CK, C], bf16)
            nc.sync.dma_start(out=wps, in_=wpd.ap().rearrange("(c p) n -> p c n", c=CK))
            w2s = cpool.tile([128, CK, K * C], bf16)
            nc.sync.dma_start(out=w2s, in_=w2d.ap().rearrange("(c p) n -> p c n", c=CK))
            m1s = cpool.tile([128, CK, 256], bf16)
            nc.sync.dma_start(out=m1s, in_=m1d.ap().rearrange("(c p) n -> p c n", c=CK))
            m2s = cpool.tile([128, 2, 256], bf16)
            nc.sync.dma_start(out=m2s, in_=m2d.ap().rearrange("(c p) n -> p c n", c=2))
            m3s = cpool.tile([128, 2, C], bf16)
            nc.sync.dma_start(out=m3s, in_=m3d.ap().rearrange("(c p) n -> p c n", c=2))
            ablkf = cpool.tile([TOK, K, TOK], f32, tag="ablkf")
            nc.sync.dma_start(out=ablkf, in_=ablkd.ap().rearrange("p (k w) -> p k w", k=K))
            ablk = cpool.tile([TOK, K, TOK], f32r, tag="ablk32")
            nc.scalar.copy(out=ablk, in_=ablkf)
            mask = cpool.tile([TOK, TOK], bf16, tag="maskb")
            nc.sync.dma_start(out=mask, in_=maskd.ap())
            ident = cpool.tile([TOK, TOK], bf16, tag="identb")
            nc.sync.dma_start(out=ident, in_=identd.ap())
            epst = cpool.tile([128, 1], f32, tag="epst")
            nc.vector.memset(epst, EPS)
            nhalf = cpool.tile([128, 1], f32, tag="nhalf")
            nc.vector.memset(nhalf, -0.5)
            ebase = cpool.tile([128, 1], f32, tag="ebase")
            nc.vector.memset(ebase, float(np.exp(float(D) ** -0.5)))

            def _bc(t, shape):
                ap = [t.ap[0][:] if False else [t.ap[0][0], shape[0]]]
                for n in shape[1:]:
                    ap.append([0, n])
                return bass.AP(tensor=t.tensor, offset=t.offset, ap=ap)
            if ln1aff:
                g1t = cpool.tile([128, J], f32, tag="g1t")
                nc.sync.dma_start(out=g1t, in_=_bcast_row_ap(g1d.ap(), 0, 128, J))
                b1t = cpool.tile([128, J], f32, tag="b1t")
                nc.sync.dma_start(out=b1t, in_=_bcast_row_ap(b1d.ap(), 0, 128, J))
            btiles = {}
            for nm, t in biasd.items():
                ln = t.shape[0]
                bt = cpool.tile([128, ln], f32, tag=f"bt_{nm}")
                nc.sync.dma_start(out=bt, in_=_bcast_row_ap(t.ap(), 0, 128, ln))
                btiles[nm] = bt
            bftiles = {}
            for nm, t in biasf.items():
                n2 = t.shape[1]
                bt = cpool.tile([128, n2], f32, tag=f"bt_{nm}")
                nc.sync.dma_start(out=bt, in_=t.ap())
                bftiles[nm] = bt

            xba = xbd.ap()
            outa = outd.ap()
            scl = float(D) ** -0.5

            def emit_loads(t0):
                """x (T layout, bf16) + xbar transpose (F layout)."""
                t0x = min(t0, ntok - 128)
                xTb = a3.tile([TOK, C], bf16, tag="xTb")
                nc.sync.dma_start(out=xTb, in_=xba[t0:t0 + TOK, :])
                xF = a3.tile([128, CK, 128], bf16, tag="xF")
                nc.sync.dma_start_transpose(out=xF, in_=xba[t0x:t0x + 128, :])
                return xTb, xF, t0 - t0x

            def emit_xsq(ld):
                xTb, xF, co = ld
                xFv = xF[:, :, co:co + TOK]
                xsq = a2.tile([128, CK, TOK], bf16, tag="xsq")
                (nc.gpsimd if XSQ_ENG == "pool" else nc.vector).tensor_mul(
                    out=xsq, in0=xFv, in1=xFv)
                return xsq

            def emit_stats(ld, xsq):
                """LN1 + lnA statistics for a tile (pipelined one tile ahead)."""
                xTb, xF, co = ld
                xFv = xF[:, :, co:co + TOK]
                xFg = xFv.rearrange("p c (b j) -> p c b j", j=J)
                s1 = sp.tile([128, CK, NB], f32, tag="s1")
                nc.vector.tensor_reduce(out=s1, in_=xFg, axis=mybir.AxisListType.X,
                                        op=ALU.add)
                s2 = sp.tile([128, CK, NB], f32, tag="s2")
                nc.vector.tensor_reduce(out=s2,
                                        in_=xsq.rearrange("p c (b j) -> p c b j", j=J),
                                        axis=mybir.AxisListType.X, op=ALU.add)
                aa = sp.tile([128, CK, NB], f32, tag="aa")
                nc.vector.scalar_tensor_tensor(out=aa, in0=s1, scalar=1.0 / (J * J),
                                               in1=s1, op0=ALU.mult, op1=ALU.mult)
                varj = sp.tile([128, CK, NB], f32, tag="varj")
                nc.vector.scalar_tensor_tensor(out=varj, in0=s2, scalar=1.0 / J,
                                               in1=aa, op0=ALU.mult, op1=ALU.subtract)
                rj = sp.tile([128, CK, NB], f32, tag="rj")
                nc.gpsimd.tensor_tensor(out=rj, in0=varj,
                                        in1=_bc(nhalf, [128, CK, NB]), op=ALU.pow)
                mrj = sp.tile([128, CK, NB], f32, tag="mrj")
                nc.vector.scalar_tensor_tensor(out=mrj, in0=s1, scalar=1.0 / J,
                                               in1=rj, op0=ALU.mult, op1=ALU.mult)
                stA = sp.tile([TOK, 6], f32, tag="stA")
                nc.vector.bn_stats(out=stA, in_=xTb)
                mvA = sp.tile([TOK, 2], f32, tag="mvA")
                nc.vector.bn_aggr(out=mvA, in_=stA)
                rA = sp.tile([TOK, 1], f32, tag="rA")
                nc.gpsimd.tensor_tensor(out=rA, in0=mvA[:, 1:2],
                                        in1=_bc(nhalf, [TOK, 1]), op=ALU.pow)
                return rj, mrj, mvA, rA

            tls = _tiles(b_core)
            loads = {0: emit_loads(tls[0][0] * J)}
            xsqs = {0: emit_xsq(loads[0])}
            stats = {0: emit_stats(loads[0], xsqs[0])}

            for ti, (b0, wb0, wnb) in enumerate(tls):
                t0 = b0 * J
                woff = (wb0 - b0) * J
                wntok = wnb * J
                # prefetch next tile's loads + stats ahead of this tile's
                # compute so the in-order queues never park them behind
                # late-phase work of the current tile
                if ti + 1 < len(tls):
                    loads[ti + 1] = emit_loads(tls[ti + 1][0] * J)
                    if PREF_STATS >= 1:
                        xsqs[ti + 1] = emit_xsq(loads[ti + 1])
                    if PREF_STATS >= 2:
                        stats[ti + 1] = emit_stats(loads[ti + 1], xsqs[ti + 1])
                xTb, xF, co = loads.pop(ti)
                if ti not in xsqs:
                    xsqs[ti] = emit_xsq(loads[ti] if ti in loads else (xTb, xF, co))
                if ti not in stats:
                    stats[ti] = emit_stats((xTb, xF, co), xsqs[ti])
                rj, mrj, mvA, rA = stats.pop(ti)
                xsqs.pop(ti, None)
                xFv = xF[:, :, co:co + TOK]
                xFg = xFv.rearrange("p c (b j) -> p c b j", j=J)

                xg = a2.tile([128, CK, TOK], bf16, tag="xg")
                xgg = xg.rearrange("p c (b j) -> p c b j", j=J)
                tmp1 = sp.tile([128, CK, TOK], bf16, tag="tmp1")
                t1g = tmp1.rearrange("p c (b j) -> p c b j", j=J)
                nc.vector.tensor_mul(out=t1g, in0=xFg,
                                     in1=rj.to_broadcast([128, CK, NB, J]))
                nc.vector.tensor_sub(out=xgg, in0=t1g,
                                     in1=mrj.to_broadcast([128, CK, NB, J]))
                if ln1aff:
                    ga = g1t
                    gb = bass.AP(tensor=ga.tensor, offset=ga.offset,
                                 ap=[ga.ap[0], [0, CK], [0, NB], ga.ap[1]])
                    ba = b1t
                    bb = bass.AP(tensor=ba.tensor, offset=ba.offset,
                                 ap=[ba.ap[0], [0, CK], [0, NB], ba.ap[1]])
                    nc.vector.tensor_mul(out=xgg, in0=xgg, in1=gb)
                    nc.vector.tensor_add(out=xgg, in0=xgg, in1=bb)

                # ---- lnA apply + xbar transpose -> xaF ----
                xa = a2.tile([128, C], bf16, tag="xa")
                nc.vector.tensor_scalar(out=xa[:TOK, :], in0=xTb, scalar1=mvA[:, 0:1],
                                        scalar2=rA, op0=ALU.subtract, op1=ALU.mult)
                if "x" in PE_TR:
                    xatp = ps_e.tile([128, CK, 120], bf16, tag="pse")
                    for c in range(CK):
                        nc.tensor.transpose(xatp[:, c, :TOK],
                                            xa[:TOK, c * 128:(c + 1) * 128], ident)
                    xaFt = a2.tile([128, CK, TOK], bf16, tag="xaFt")
                    nc.vector.tensor_copy(out=xaFt, in_=xatp[:, :, :TOK])
                    xaF = xaFt
                else:
                    xaFt = a2.tile([128, CK, 128], bf16, tag="xaFt")
                    nc.sync.dma_start_transpose(out=xaFt, in_=xa)
                    xaF = xaFt[:, :, :TOK]

                # ---- GCN1: k-loop channel matmul + adjacency ----
                xg1p = ps_b.tile([TOK, C], f32, tag="psb")
                for k in range(K):
                    y1p = ps_a.tile([TOK, C], f32, tag="psa")
                    for c in range(CK):
                        nc.tensor.matmul(y1p, xg[:, c, :],
                                         w1s[:, c, k * C:(k + 1) * C],
                                         start=(c == 0), stop=(c == CK - 1))
                    y1s = sp.tile([TOK, C], f32r, tag=f"y1s{k}")
                    if "b_1g" in bias_on:
                        nc.vector.tensor_add(out=y1s, in0=y1p,
                                             in1=btiles["b_1g"][:TOK, k * C:(k + 1) * C])
                    elif _copy_on_act(CMIX, "y1", k):
                        nc.scalar.copy(out=y1s, in_=y1p)
                    else:
                        nc.vector.tensor_copy(out=y1s, in_=y1p)
                    nc.tensor.matmul(xg1p, ablk[:, k, :], y1s,
                                     start=(k == 0), stop=(k == K - 1))

                # xg1 -> bf16 -> xbar transpose -> xg1F
                xg1c = a2.tile([128, C], bf16, tag="xg1c")
                if PIN_FOLD:
                    nc.vector.tensor_scalar_mul(out=xg1c[:TOK, :], in0=xg1p,
                                                scalar1=0.5)
                elif XG1C_ACT:
                    nc.scalar.copy(out=xg1c[:TOK, :], in_=xg1p)
                else:
                    nc.vector.tensor_copy(out=xg1c[:TOK, :], in_=xg1p)
                xg1Ft = a2.tile([128, CK, 128], bf16, tag="xg1Ft")
                nc.sync.dma_start_transpose(out=xg1Ft, in_=xg1c)
                xg1F = xg1Ft[:, :, :TOK]

                # ---- qkv: q,k ----
                if QK_FT:
                    qkTc = a2.tile([128, 2 * C], bf16, tag="qkTc")
                    for g in range(2):
                        qkp = ps_c.tile([TOK, C], f32, tag="psc")
                        for c in range(CK):
                            nc.tensor.matmul(qkp, xaF[:, c, :],
                                             wqks[:, c, g * C:(g + 1) * C],
                                             start=(c == 0), stop=(c == CK - 1))
                        dst = qkTc[:TOK, g * C:(g + 1) * C]
                        if "b_qkv" in bias_on:
                            nc.vector.tensor_add(out=dst, in0=qkp,
                                                 in1=btiles["b_qkv"][:TOK, g * C:(g + 1) * C])
                        elif g == 0:
                            nc.vector.tensor_copy(out=dst, in_=qkp)
                        else:
                            nc.scalar.copy(out=dst, in_=qkp)
                    qkFt = a2.tile([128, 8, 128], bf16, tag="qkFt")
                    nc.sync.dma_start_transpose(out=qkFt, in_=qkTc)
                    qkF = qkFt[:, :, :TOK]
                else:
                    qkFf = a2.tile([128, 8, TOK], bf16, tag="qkF")
                    for g in range(2):
                        qkp = ps_c.tile([128, 4, TOK], f32, tag="psc")
                        for d in range(4):
                            dd = g * 4 + d
                            for c in range(CK):
                                nc.tensor.matmul(qkp[:, d, :], wqks[:, c, dd * 128:(dd + 1) * 128],
                                                 xaF[:, c, :], start=(c == 0), stop=(c == CK - 1))
                        dst = qkFf[:, g * 4:(g + 1) * 4, :]
                        if "b_qkv" in bias_on:
                            bq = bftiles["bf_qk"]
                            bqa = bass.AP(tensor=bq.tensor,
                                          offset=bq.offset + g * 4,
                                          ap=[bq.ap[0], [1, 4], [0, TOK]])
                            nc.vector.tensor_add(out=dst, in0=qkp, in1=bqa)
                        elif g == 0:
                            nc.vector.tensor_copy(out=dst, in_=qkp)
                        else:
                            nc.scalar.copy(out=dst, in_=qkp)
                    qkF = qkFf

                # ---- v: F->T ----
                if VP_POOL == "c":
                    vp = ps_c.tile([TOK, C], f32, tag="psc")
                else:
                    vp = ps_a.tile([TOK, C], f32, tag="psa")
                for c in range(CK):
                    nc.tensor.matmul(vp, xaF[:, c, :], wvs[:, c, :],
                                     start=(c == 0), stop=(c == CK - 1))
                vsb = a2.tile([TOK, H, 65], bf16, tag="vsb")
                nc.vector.memset(vsb[:, :, 64:65], 1.0)
                vdst = vsb[:, :, 0:64]
                vsrc = vp.rearrange("p (h d) -> p h d", h=H)
                if "b_qkv" in bias_on:
                    bv = btiles["b_qkv"][:TOK, 2 * C:3 * C].rearrange(
                        "p (h d) -> p h d", h=H)
                    nc.vector.tensor_add(out=vdst, in0=vsrc, in1=bv)
                elif _copy_on_act(CMIX, "vsb", 0):
                    nc.scalar.copy(out=vdst, in_=vsrc)
                else:
                    nc.vector.tensor_copy(out=vdst, in_=vsrc)

                # ---- attention: halves = even heads (part 0-63) / odd ----
                oF = a2.tile([128, CK, TOK], bf16, tag="oF")
                for half in range(2):
                    p0 = half * 64
                    scp = ps_c.tile([TOK, 4, TOK], f32, tag="psc")
                    for j in range(4):
                        nc.tensor.matmul(scp[:, j, :], qkF[p0:p0 + 64, 4 + j, :],
                                         qkF[p0:p0 + 64, j, :], start=True, stop=True)
                    scs = a2.tile([TOK, 4, TOK], f32, tag=f"scs{half}")
                    if SCS_DVE and half == 1:
                        nc.vector.tensor_copy(out=scs, in_=scp)
                    else:
                        nc.scalar.copy(out=scs, in_=scp)
                    U = a2.tile([TOK, 4, TOK], bf16, tag=f"U{half}")
                    nc.gpsimd.tensor_tensor(out=U, in0=_bc(ebase, [TOK, 4, TOK]),
                                            in1=scs, op=ALU.pow)
                    mb = bass.AP(tensor=mask.tensor, offset=mask.offset,
                                 ap=[mask.ap[0], [0, 4], mask.ap[1]])
                    (nc.gpsimd if MASK_ENG == "pool" else nc.vector).tensor_mul(
                        out=U, in0=U, in1=mb)
                    ozp = ps_c.tile([65, 4, TOK], f32, tag="psc")
                    for j in range(4):
                        h = 2 * j + half
                        nc.tensor.matmul(ozp[:, j, :], vsb[:, h, :], U[:, j, :],
                                         start=True, stop=True)
                    rz = sp.tile([1, 4, TOK], bf16, tag=f"rz{half}")
                    with nc.allow_low_precision(reason="bf16 softmax recip ok"):
                        nc.vector.reciprocal(out=rz, in_=ozp[64:65, :, :])
                    rzb = sp.tile([64, 4, TOK], bf16, tag=f"rzb{half}")
                    nc.gpsimd.partition_broadcast(rzb, rz)
                    if half == 0:
                        nc.vector.tensor_mul(out=oF[p0:p0 + 64, :, :],
                                             in0=ozp[0:64, :, :], in1=rzb)
                    else:
                        nc.vector.tensor_mul(out=oF[p0:p0 + 64, :, :],
                                             in0=ozp[0:64, :, :], in1=rzb)

                # ---- pin/gin (F layout) ----
                xg1Fv = xg1F
                if PIN_FOLD:
                    # xg1F already holds 0.5*xg1; gin = (2.5*xg1F_half + oF),
                    # with the 0.8 folded into w2 on the host.
                    gin = a2.tile([128, CK, TOK], bf16, tag="gin")
                    nc.vector.scalar_tensor_tensor(out=gin, in0=xg1Fv, scalar=2.5,
                                                   in1=oF, op0=ALU.mult, op1=ALU.add)
                else:
                    pin = a2.tile([128, CK, TOK], bf16, tag="pin")
                    nc.vector.scalar_tensor_tensor(out=pin, in0=xg1Fv, scalar=0.5,
                                                   in1=oF, op0=ALU.mult, op1=ALU.add)
                    gin = a2.tile([128, CK, TOK], bf16, tag="gin")
                    nc.vector.scalar_tensor_tensor(out=gin, in0=oF, scalar=0.8,
                                                   in1=xg1Fv, op0=ALU.mult, op1=ALU.add)

                # ---- proj (+ gcn2 adjacency accumulated or split) ----
                xap = ps_b.tile([TOK, C], f32, tag="psb")
                if PIN_FOLD:
                    for c in range(CK):
                        nc.tensor.matmul(xap, oF[:, c, :], wps[:, c, :],
                                         start=(c == 0), stop=False)
                    for c in range(CK):
                        nc.tensor.matmul(xap, xg1Fv[:, c, :], wps[:, c, :],
                                         start=False, stop=SPLIT_XAP and c == CK - 1)
                else:
                    for c in range(CK):
                        nc.tensor.matmul(xap, pin[:, c, :], wps[:, c, :],
                                         start=(c == 0), stop=SPLIT_XAP and c == CK - 1)

                yT = a3.tile([TOK, C], f32, tag="yT")
                if SPLIT_XAP:
                    nc.vector.tensor_add(out=yT, in0=xap, in1=xTb)
                if SPLIT_XAP:
                    xg2p = ps_b.tile([TOK, C], f32, tag="psb")
                else:
                    xg2p = xap
                for k in range(K):
                    y2p = ps_e.tile([TOK, C], f32, tag="pse")
                    for c in range(CK):
                        nc.tensor.matmul(y2p, gin[:, c, :],
                                         w2s[:, c, k * C:(k + 1) * C],
                                         start=(c == 0), stop=(c == CK - 1))
                    y2s = sp.tile([TOK, C], f32r, tag=f"y2s{k}")
                    if "b_2g" in bias_on:
                        nc.vector.tensor_add(out=y2s, in0=y2p,
                                             in1=btiles["b_2g"][:TOK, k * C:(k + 1) * C])
                    elif _copy_on_act(CMIX, "y2", k):
                        nc.scalar.copy(out=y2s, in_=y2p)
                    else:
                        nc.vector.tensor_copy(out=y2s, in_=y2p)
                    nc.tensor.matmul(xg2p, ablk[:, k, :], y2s,
                                     start=SPLIT_XAP and k == 0, stop=(k == K - 1))

                # ---- y = x + proj + gcn2 [+ proj bias] ----
                if SPLIT_XAP:
                    nc.vector.tensor_add(out=yT, in0=xg2p, in1=yT)
                else:
                    nc.vector.tensor_add(out=yT, in0=xap, in1=xTb)
                if "b_p" in bias_on:
                    nc.vector.tensor_add(out=yT, in0=yT, in1=btiles["b_p"][:TOK, :])

                # ---- LN2 + transpose ----
                st2 = sp.tile([TOK, 6], f32, tag="st2")
                nc.vector.bn_stats(out=st2, in_=yT)
                mv2 = sp.tile([TOK, 2], f32, tag="mv2")
                nc.vector.bn_aggr(out=mv2, in_=st2)
                r2 = sp.tile([TOK, 1], f32, tag="r2")
                nc.gpsimd.tensor_tensor(out=r2, in0=mv2[:, 1:2],
                                        in1=_bc(nhalf, [TOK, 1]), op=ALU.pow)
                z = a2.tile([128, C], bf16, tag="z")
                nc.vector.tensor_scalar(out=z[:TOK, :], in0=yT, scalar1=mv2[:, 0:1],
                                        scalar2=r2, op0=ALU.subtract, op1=ALU.mult)
                if "z" in PE_TR:
                    ztp = ps_e.tile([128, CK, 120], bf16, tag="pse")
                    for c in range(CK):
                        nc.tensor.transpose(ztp[:, c, :TOK],
                                            z[:TOK, c * 128:(c + 1) * 128], ident)
                    zFt = a2.tile([128, CK, TOK], bf16, tag="zFt")
                    nc.scalar.copy(out=zFt, in_=ztp[:, :, :TOK])
                    zFv = zFt
                else:
                    zFt = a2.tile([128, CK, 128], bf16, tag="zFt")
                    nc.sync.dma_start_transpose(out=zFt, in_=z)
                    zFv = zFt[:, :, :TOK]

                # ---- MLP: m1 F->F, m2 F->F, m3 F->T ----
                h1p = ps_d.tile([128, 2, TOK], f32, tag="psd")
                for d in range(2):
                    for c in range(CK):
                        nc.tensor.matmul(h1p[:, d, :], m1s[:, c, d * 128:(d + 1) * 128],
                                         zFv[:, c, :], start=(c == 0), stop=(c == CK - 1))
                h1F = a2.tile([128, 2, TOK], bf16, tag="h1F")
                if "b_m1" in bias_on:
                    bm1 = bftiles["bf_m1"]
                    for d in range(2):
                        nc.scalar.activation(out=h1F[:, d, :], in_=h1p[:, d, :],
                                             func=AF.Gelu,
                                             bias=bm1[:, d:d + 1], scale=1.0)
                else:
                    nc.scalar.activation(out=h1F, in_=h1p, func=AF.Gelu)

                h2p = ps_d.tile([128, 2, TOK], f32, tag="psd")
                for d in range(2):
                    for c in range(2):
                        nc.tensor.matmul(h2p[:, d, :], m2s[:, c, d * 128:(d + 1) * 128],
                                         h1F[:, c, :], start=(c == 0), stop=(c == 1))
                g2F = a2.tile([128, 2, TOK], bf16, tag="g2F")
                if "b_m2" in bias_on:
                    bm2 = bftiles["bf_m2"]
                    for d in range(2):
                        nc.scalar.activation(out=g2F[:, d, :], in_=h2p[:, d, :],
                                             func=AF.Gelu,
                                             bias=bm2[:, d:d + 1], scale=1.0)
                else:
                    nc.scalar.activation(out=g2F, in_=h2p, func=AF.Gelu)
                h2F = a2.tile([128, 2, TOK], bf16, tag="h2F")
                nc.vector.tensor_add(out=h2F, in0=g2F, in1=h1F)

                if H3_POOL == "d":
                    h3p = ps_d.tile([TOK, C], f32, tag="psd")
                else:
                    h3p = ps_b.tile([TOK, C], f32, tag="psb")
                for c in range(2):
                    nc.tensor.matmul(h3p, h2F[:, c, :], m3s[:, c, :],
                                     start=(c == 0), stop=(c == 1))
                g3 = sp.tile([TOK, C], f32, tag="g3")
                if "b_m3" in bias_on:
                    tb3 = sp.tile([TOK, C], f32, tag="tb3")
                    nc.vector.tensor_add(out=tb3, in0=h3p, in1=btiles["b_m3"][:TOK, :])
                    nc.scalar.activation(out=g3, in_=tb3, func=AF.Gelu)
                else:
                    nc.scalar.activation(out=g3, in_=h3p, func=AF.Gelu)
                outT = a3.tile([TOK, C], f32, tag="outT")
                nc.vector.tensor_add(out=outT, in0=g3, in1=yT)

                nc.sync.dma_start(out=outa[t0 + woff:t0 + woff + wntok, :],
                                  in_=outT[woff:woff + wntok, :])

    nc.compile()
    return nc


def _is_ones(a):
    return bool(np.all(a == 1.0))


def _is_zeros(a):
    return bool(np.all(a == 0.0))


def _prep(inputs):
    """Host-side folds and layout transforms. Returns (flags, shared arrays)."""
    adj = inputs["adj"].astype(np.float32)
    f64 = np.float64

    lnA_g, lnA_b = inputs["lnA_g"], inputs["lnA_b"]
    qkv_w = inputs["qkv_w"].astype(f64)
    wqkv = (qkv_w * lnA_g.astype(f64)[None, :])
    bqkv = inputs["qkv_b"].astype(f64) + qkv_w @ lnA_b.astype(f64)

    ln2_g, ln2_b = inputs["ln2_g"], inputs["ln2_b"]
    m1_w = inputs["m1_w"].astype(f64)
    wm1 = m1_w * ln2_g.astype(f64)[None, :]
    bm1 = inputs["m1_b"].astype(f64) + m1_w @ ln2_b.astype(f64)

    flags = set()
    if not (_is_ones(inputs["ln1_g"]) and _is_zeros(inputs["ln1_b"])):
        flags.add("ln1aff")

    bf = ml_dtypes.bfloat16
    wqkvT = wqkv.astype(np.float32).T  # [C, 3C]
    shared = {
        "w1b": np.ascontiguousarray(inputs["gcn1_w"].astype(np.float32).T).astype(bf),
        "wqkb": np.ascontiguousarray(wqkvT[:, 0:2 * C]).astype(bf),
        "wvb": np.ascontiguousarray(wqkvT[:, 2 * C:3 * C]).astype(bf),
        "wpb": np.ascontiguousarray(inputs["proj_w"].astype(np.float32).T).astype(bf),
        "w2b": np.ascontiguousarray(
            (0.8 if PIN_FOLD else 1.0) * inputs["gcn2_w"].astype(np.float32).T
        ).astype(bf),
        "m1b": np.ascontiguousarray(wm1.astype(np.float32).T).astype(bf),
        "m2b": np.ascontiguousarray(inputs["m2_w"].astype(np.float32).T).astype(bf),
        "m3b": np.ascontiguousarray(inputs["m3_w"].astype(np.float32).T).astype(bf),
        "g1d": inputs["ln1_g"].astype(np.float32),
        "b1d": inputs["ln1_b"].astype(np.float32),
    }
    ablk = np.zeros((TOK, K, TOK), np.float32)
    for k in range(K):
        for b in range(NB):
            ablk[b * J:(b + 1) * J, k, b * J:(b + 1) * J] = adj[k]
    shared["ablk"] = ablk.reshape(TOK, K * TOK)
    m = np.zeros((TOK, TOK), np.float32)
    for b in range(NB):
        m[b * J:(b + 1) * J, b * J:(b + 1) * J] = 1.0
    shared["maskd"] = m.astype(bf)
    shared["identd"] = np.eye(TOK, dtype=np.float32).astype(bf)

    for nm, arr in [("b_qkv", bqkv.astype(np.float32)),
                    ("b_p", inputs["proj_b"].astype(np.float32)),
                    ("b_1g", inputs["gcn1_b"].astype(np.float32)),
                    ("b_2g", inputs["gcn2_b"].astype(np.float32)),
                    ("b_m1", bm1.astype(np.float32)),
                    ("b_m2", inputs["m2_b"].astype(np.float32)),
                    ("b_m3", inputs["m3_b"].astype(np.float32))]:
        if not _is_zeros(arr):
            flags.add(nm)
            shared[nm] = arr
    if "b_qkv" in flags:
        shared["bf_qk"] = np.ascontiguousarray(
            bqkv[:2 * C].astype(np.float32).reshape(8, 128).T)
    if "b_m1" in flags:
        shared["bf_m1"] = np.ascontiguousarray(
            bm1.astype(np.float32).reshape(2, 128).T)
    if "b_m2" in flags:
        shared["bf_m2"] = np.ascontiguousarray(
            inputs["m2_b"].astype(np.float32).reshape(2, 128).T)
    return frozenset(flags), shared


def kernel(**inputs):
    flags, shared = _prep(inputs)
    key = (B_CORE, flags)
    if key not in _CACHE:
        _CACHE[key] = _build(B_CORE, flags)
    nc = _CACHE[key]

    xb = np.ascontiguousarray(inputs["x"], dtype=np.float32) \
        .astype(ml_dtypes.bfloat16)
    in_maps = []
    for c in range(N_CORES):
        m = dict(shared)
        m["xb"] = xb[c * B_CORE:(c + 1) * B_CORE].reshape(B_CORE * J, C)
        in_maps.append(m)

    res = bass_utils.run_bass_kernel_spmd(nc, in_maps, core_ids=list(range(N_CORES)))
    outs = [res.results[c]["out"].reshape(B_CORE, J, C) for c in range(N_CORES)]
    return np.concatenate(outs, axis=0)
